# revision 6
# baseline (speedup 1.0000x reference)
"""Trainium2 Bass kernel for nn_MinEuclideanDistBlock.

Math (reference):
  x: (B=64, C=3, L=2048), shapelets: (C=3, N=256, S=64)
  W = L - S + 1 = 1985 sliding windows
  d2[b,c,w,n] = |win|^2 + |shp|^2 - 2 win.shp    (win = x[b,c,w:w+S])
  d = sqrt(max(d2, 0));  out[b,0,n] = min_w sum_c d[b,c,w,n]

Device strategy (per core, batch-sharded B/8 = 8 batches per core):
  - For each (b, c): build a "shifted" SBUF tile T[65, 2048]:
      T[s, w] = x[b, c, w+s] for s in [0,64)   (one DMA, overlapping AP)
      T[64, w] = win_sq[w] = sum_s x[b,c,w+s]^2 (computed on-chip via
      log2 shifted adds on DVE, row-copied via DMA)
  - lhsT[k, n] (host precomputed): rows 0..63 = -2*shapelets^T, row 64 = 1.
    One float32r matmul per 512-wide w chunk gives
      psum[n, w] = -2*cross + win_sq[w]
  - ACT: d = sqrt(psum + bias) with per-partition bias = shp_sq[n].
  - DVE: accumulate channels, then tensor_tensor_reduce(min over w).
"""

import numpy as np

S = 64
NSH = 256
C = 3
B = 64
L = 2048
W = L - S + 1  # 1985
NCORES = 8
BPC = B // NCORES  # 8
NT = 2  # shapelet tiles of 128
WCHUNKS = [(0, 512), (512, 512), (1024, 512), (1536, W - 1536)]

_cache = {}


def _build_nc():
    from contextlib import ExitStack  # noqa: F401

    import concourse.bass as bass
    import concourse.bacc as bacc
    import concourse.mybir as mybir
    import concourse.tile as tile

    f32 = mybir.dt.float32
    bf16 = mybir.dt.bfloat16
    AF = mybir.ActivationFunctionType
    ALU = mybir.AluOpType

    nc = bacc.Bacc()
    xs = nc.dram_tensor("xs", [BPC, C, L], bf16, kind="ExternalInput")
    wts = nc.dram_tensor("wts", [C, NT, S + 1, 128], bf16, kind="ExternalInput")
    ssq = nc.dram_tensor("ssq", [C, NT, 128], f32, kind="ExternalInput")
    out = nc.dram_tensor("out", [BPC, NT, 128], f32, kind="ExternalOutput")

    with tile.TileContext(nc) as tc:
        with (
            tc.tile_pool(name="consts", bufs=1) as consts,
            tc.tile_pool(name="prep", bufs=1) as prep,
            tc.tile_pool(name="tpool", bufs=3) as tpool,
            tc.tile_pool(name="psump", bufs=2, space="PSUM") as psump,
            tc.tile_pool(name="accp", bufs=2) as accp,
            tc.tile_pool(name="tmpp", bufs=3) as tmpp,
            tc.tile_pool(name="minvp", bufs=8) as minvp,
        ):
            # ---- constants ----
            w_all = consts.tile([S + 1, C * NT * 128], bf16)
            biases = {}
            for c in range(C):
                for nt in range(NT):
                    idx = c * NT + nt
                    nc.sync.dma_start(
                        out=w_all[:, idx * 128 : (idx + 1) * 128],
                        in_=wts[c, nt, :, :],
                    )
                    bt = consts.tile([128, 1], f32, name=f"bias_{c}_{nt}")
                    nc.sync.dma_start(out=bt, in_=ssq[c, nt, :])
                    biases[(c, nt)] = bt

            # ---- prep: win_sq for all 24 (b, c) rows ----
            xr = prep.tile([BPC * C, L], bf16)
            nc.sync.dma_start(out=xr, in_=xs[:, :, :])
            xsq = prep.tile([BPC * C, L], f32)
            nc.vector.tensor_mul(xsq, xr, xr)
            msA = prep.tile([BPC * C, L], f32)
            msB = prep.tile([BPC * C, L], f32)
            winsq = prep.tile([BPC * C, L], bf16)
            bufs = [xsq, msA, msB]
            srci = 0
            cnt = L
            for step_i, k in enumerate([1, 2, 4, 8, 16, 32]):
                src = bufs[srci]
                cnt = cnt - k
                if k == 32:  # last step writes the bf16 win_sq row directly
                    dst = winsq
                else:
                    dst = bufs[1 + (step_i % 2)]
                nc.vector.tensor_add(
                    dst[:, 0:cnt], src[:, 0:cnt], src[:, k : k + cnt]
                )
                srci = 1 + (step_i % 2)
            assert cnt == W

            # ---- main loop ----
            accs = {}
            for b in range(BPC):
                for c in range(C):
                    bc = b * C + c
                    T = tpool.tile([S + 1, L], bf16, name="T")
                    base = xs[b, c, :]
                    apov = bass.AP(
                        tensor=base.tensor,
                        offset=base.offset,
                        ap=[[1, S], [1, W]],
                    )
                    nc.sync.dma_start(out=T[0:S, 0:W], in_=apov)
                    nc.sync.dma_start(
                        out=T[S : S + 1, 0:W], in_=winsq[bc : bc + 1, 0:W]
                    )
                    for nt in range(NT):
                        idx = c * NT + nt
                        lhsT = w_all[:, idx * 128 : (idx + 1) * 128]
                        ps = psump.tile([128, 2048], f32, name="ps")
                        for w0, wl in WCHUNKS:
                            nc.tensor.matmul(
                                ps[:, w0 : w0 + wl],
                                lhsT=lhsT,
                                rhs=T[:, w0 : w0 + wl],
                                start=True,
                                stop=True,
                            )
                        if c == 0:
                            acc = accp.tile([128, W], f32, name=f"acc{nt}")
                            accs[nt] = acc
                            nc.scalar.activation(
                                acc, ps[:, 0:W], AF.Sqrt, bias=biases[(c, nt)]
                            )
                        elif c == 1:
                            tmp = tmpp.tile([128, W], f32, name="tmp")
                            nc.scalar.activation(
                                tmp, ps[:, 0:W], AF.Sqrt, bias=biases[(c, nt)]
                            )
                            nc.vector.tensor_add(accs[nt], accs[nt], tmp)
                        else:
                            tmp = tmpp.tile([128, W], f32, name="tmp")
                            nc.scalar.activation(
                                tmp, ps[:, 0:W], AF.Sqrt, bias=biases[(c, nt)]
                            )
                            scratch = tmpp.tile([128, W], f32, name="scratch")
                            minv = minvp.tile([128, 1], f32, name="minv")
                            nc.vector.tensor_add(scratch, accs[nt], tmp)
                            nc.vector.tensor_reduce(
                                minv,
                                scratch,
                                mybir.AxisListType.X,
                                ALU.min,
                            )
                            nc.sync.dma_start(out=out[b, nt, :], in_=minv)
    nc.compile()
    return nc


def _get_nc():
    if "nc" not in _cache:
        _cache["nc"] = _build_nc()
    return _cache["nc"]


def _prep_inputs(x, shapelets):
    import ml_dtypes

    bf16 = ml_dtypes.bfloat16
    x = np.ascontiguousarray(np.asarray(x), dtype=np.float32)
    sh = np.asarray(shapelets, dtype=np.float32)
    # round shapelets to bf16 once; all derived quantities use the rounded
    # values so d2 stays an exact squared distance of the rounded vectors
    shb = sh.astype(bf16).astype(np.float32)
    shT = np.transpose(shb, (0, 2, 1))  # (C, S, N)
    wts = np.empty((C, NT, S + 1, 128), np.float32)
    for nt in range(NT):
        wts[:, nt, :S, :] = -2.0 * shT[:, :, nt * 128 : (nt + 1) * 128]
    wts[:, :, S, :] = 1.0
    ssq = np.sum(shb * shb, axis=2).reshape(C, NT, 128).astype(np.float32)
    wts_b = np.ascontiguousarray(wts.astype(bf16))
    ssq = np.ascontiguousarray(ssq)
    xb = x.astype(bf16)
    in_maps = [
        {
            "xs": np.ascontiguousarray(xb[k * BPC : (k + 1) * BPC]),
            "wts": wts_b,
            "ssq": ssq,
        }
        for k in range(NCORES)
    ]
    return in_maps


def _gather(results):
    outs = [np.asarray(r["out"]).reshape(BPC, NSH) for r in results]
    full = np.concatenate(outs, axis=0)  # (64, 256)
    return np.ascontiguousarray(full[:, None, :]).astype(np.float32)  # (64, 1, 256)


def kernel(x, shapelets):
    from concourse.bass_utils import run_bass_kernel_spmd

    nc = _get_nc()
    in_maps = _prep_inputs(x, shapelets)
    res = run_bass_kernel_spmd(nc, in_maps, core_ids=list(range(NCORES)))
    return _gather(res.results)


def kernel_traced(x, shapelets):
    """Like kernel() but requests an NTFF trace; returns (out, BassKernelResults)."""
    from concourse.bass_utils import run_bass_kernel_spmd

    nc = _get_nc()
    in_maps = _prep_inputs(x, shapelets)
    res = run_bass_kernel_spmd(nc, in_maps, core_ids=list(range(NCORES)), trace=True)
    return _gather(res.results), res


# revision 9
# speedup vs baseline: 1.0072x; 1.0072x over previous
"""Trainium2 Bass kernel for nn_MinEuclideanDistBlock.

Math (reference):
  x: (B=64, C=3, L=2048), shapelets: (C=3, N=256, S=64)
  W = L - S + 1 = 1985 sliding windows
  d2[b,c,w,n] = |win|^2 + |shp|^2 - 2 win.shp    (win = x[b,c,w:w+S])
  d = sqrt(max(d2, 0));  out[b,0,n] = min_w sum_c d[b,c,w,n]

Device strategy (per core, batch-sharded B/8 = 8 batches per core):
  - For each (b, c): build a "shifted" SBUF tile T[65, 2048]:
      T[s, w] = x[b, c, w+s] for s in [0,64)   (one DMA, overlapping AP)
      T[64, w] = win_sq[w] = sum_s x[b,c,w+s]^2 (computed on-chip via
      log2 shifted adds on DVE, row-copied via DMA)
  - lhsT[k, n] (host precomputed): rows 0..63 = -2*shapelets^T, row 64 = 1.
    One float32r matmul per 512-wide w chunk gives
      psum[n, w] = -2*cross + win_sq[w]
  - ACT: d = sqrt(psum + bias) with per-partition bias = shp_sq[n].
  - DVE: accumulate channels, then tensor_tensor_reduce(min over w).
"""

import numpy as np

S = 64
NSH = 256
C = 3
B = 64
L = 2048
W = L - S + 1  # 1985
NCORES = 8
BPC = B // NCORES  # 8
NT = 2  # shapelet tiles of 128
WCHUNKS = [(0, 512), (512, 512), (1024, 512), (1536, W - 1536)]

_cache = {}


def _build_nc(reps=1):
    from contextlib import ExitStack  # noqa: F401

    import concourse.bass as bass
    import concourse.bacc as bacc
    import concourse.mybir as mybir
    import concourse.tile as tile

    f32 = mybir.dt.float32
    bf16 = mybir.dt.bfloat16
    AF = mybir.ActivationFunctionType
    ALU = mybir.AluOpType

    nc = bacc.Bacc()
    xs = nc.dram_tensor("xs", [BPC, C, L], bf16, kind="ExternalInput")
    wts = nc.dram_tensor("wts", [C, NT, S + 1, 128], bf16, kind="ExternalInput")
    ssq = nc.dram_tensor("ssq", [C, NT, 128], f32, kind="ExternalInput")
    out = nc.dram_tensor("out", [BPC, NT, 128], f32, kind="ExternalOutput")

    with tile.TileContext(nc) as tc:
        with (
            tc.tile_pool(name="consts", bufs=1) as consts,
            tc.tile_pool(name="prep", bufs=1) as prep,
            tc.tile_pool(name="tpool", bufs=3) as tpool,
            tc.tile_pool(name="psump", bufs=2, space="PSUM") as psump,
            tc.tile_pool(name="accp", bufs=2) as accp,
            tc.tile_pool(name="tmpp", bufs=3) as tmpp,
            tc.tile_pool(name="minvp", bufs=8) as minvp,
        ):
            # ---- constants ----
            w_all = consts.tile([S + 1, C * NT * 128], bf16)
            biases = {}
            for c in range(C):
                for nt in range(NT):
                    idx = c * NT + nt
                    nc.sync.dma_start(
                        out=w_all[:, idx * 128 : (idx + 1) * 128],
                        in_=wts[c, nt, :, :],
                    )
                    bt = consts.tile([128, 1], f32, name=f"bias_{c}_{nt}")
                    nc.sync.dma_start(out=bt, in_=ssq[c, nt, :])
                    biases[(c, nt)] = bt

            # ---- prep: win_sq for all 24 (b, c) rows ----
            for _rep in range(reps):
                _body(nc, tc, bass, mybir, prep, tpool, psump, accp, tmpp, minvp,
                      xs, out, w_all, biases)
    nc.compile()
    return nc


def _body(nc, tc, bass, mybir, prep, tpool, psump, accp, tmpp, minvp,
          xs, out, w_all, biases):
    f32 = mybir.dt.float32
    bf16 = mybir.dt.bfloat16
    AF = mybir.ActivationFunctionType
    ALU = mybir.AluOpType
    if True:
        if True:
            xr = prep.tile([BPC * C, L], bf16)
            nc.sync.dma_start(out=xr, in_=xs[:, :, :])
            xsq = prep.tile([BPC * C, L], f32)
            nc.vector.tensor_mul(xsq, xr, xr)
            msA = prep.tile([BPC * C, L], f32)
            msB = prep.tile([BPC * C, L], f32)
            winsq = prep.tile([BPC * C, L], bf16)
            bufs = [xsq, msA, msB]
            srci = 0
            cnt = L
            for step_i, k in enumerate([1, 2, 4, 8, 16, 32]):
                src = bufs[srci]
                cnt = cnt - k
                if k == 32:  # last step writes the bf16 win_sq row directly
                    dst = winsq
                else:
                    dst = bufs[1 + (step_i % 2)]
                nc.vector.tensor_add(
                    dst[:, 0:cnt], src[:, 0:cnt], src[:, k : k + cnt]
                )
                srci = 1 + (step_i % 2)
            assert cnt == W

            # ---- main loop ----
            accs = {}
            for b in range(BPC):
                for c in range(C):
                    bc = b * C + c
                    T = tpool.tile([S + 1, L], bf16, name="T")
                    base = xs[b, c, :]
                    apov = bass.AP(
                        tensor=base.tensor,
                        offset=base.offset,
                        ap=[[1, S], [1, W]],
                    )
                    nc.sync.dma_start(out=T[0:S, 0:W], in_=apov)
                    nc.sync.dma_start(
                        out=T[S : S + 1, 0:W], in_=winsq[bc : bc + 1, 0:W]
                    )
                    for nt in range(NT):
                        idx = c * NT + nt
                        lhsT = w_all[:, idx * 128 : (idx + 1) * 128]
                        ps = psump.tile([128, 2048], f32, name="ps")
                        for w0, wl in WCHUNKS:
                            nc.tensor.matmul(
                                ps[:, w0 : w0 + wl],
                                lhsT=lhsT,
                                rhs=T[:, w0 : w0 + wl],
                                start=True,
                                stop=True,
                            )
                        if c == 0:
                            acc = accp.tile([128, W], f32, name=f"acc{nt}")
                            accs[nt] = acc
                            nc.scalar.activation(
                                acc, ps[:, 0:W], AF.Sqrt, bias=biases[(c, nt)]
                            )
                        elif c == 1:
                            tmp = tmpp.tile([128, W], f32, name="tmp")
                            nc.scalar.activation(
                                tmp, ps[:, 0:W], AF.Sqrt, bias=biases[(c, nt)]
                            )
                            nc.vector.tensor_add(accs[nt], accs[nt], tmp)
                        else:
                            tmp = tmpp.tile([128, W], f32, name="tmp")
                            nc.scalar.activation(
                                tmp, ps[:, 0:W], AF.Sqrt, bias=biases[(c, nt)]
                            )
                            scratch = tmpp.tile([128, W], f32, name="scratch")
                            minv = minvp.tile([128, 1], f32, name="minv")
                            nc.vector.tensor_add(scratch, accs[nt], tmp)
                            nc.vector.tensor_reduce(
                                minv,
                                scratch,
                                mybir.AxisListType.X,
                                ALU.min,
                            )
                            nc.sync.dma_start(out=out[b, nt, :], in_=minv)


def _get_nc():
    if "nc" not in _cache:
        _cache["nc"] = _build_nc()
    return _cache["nc"]


def _prep_inputs(x, shapelets):
    import ml_dtypes

    bf16 = ml_dtypes.bfloat16
    x = np.ascontiguousarray(np.asarray(x), dtype=np.float32)
    sh = np.asarray(shapelets, dtype=np.float32)
    # round shapelets to bf16 once; all derived quantities use the rounded
    # values so d2 stays an exact squared distance of the rounded vectors
    shb = sh.astype(bf16).astype(np.float32)
    shT = np.transpose(shb, (0, 2, 1))  # (C, S, N)
    wts = np.empty((C, NT, S + 1, 128), np.float32)
    for nt in range(NT):
        wts[:, nt, :S, :] = -2.0 * shT[:, :, nt * 128 : (nt + 1) * 128]
    wts[:, :, S, :] = 1.0
    ssq = np.sum(shb * shb, axis=2).reshape(C, NT, 128).astype(np.float32)
    wts_b = np.ascontiguousarray(wts.astype(bf16))
    ssq = np.ascontiguousarray(ssq)
    xb = x.astype(bf16)
    in_maps = [
        {
            "xs": np.ascontiguousarray(xb[k * BPC : (k + 1) * BPC]),
            "wts": wts_b,
            "ssq": ssq,
        }
        for k in range(NCORES)
    ]
    return in_maps


def _gather(results):
    outs = [np.asarray(r["out"]).reshape(BPC, NSH) for r in results]
    full = np.concatenate(outs, axis=0)  # (64, 256)
    return np.ascontiguousarray(full[:, None, :]).astype(np.float32)  # (64, 1, 256)


def kernel(x, shapelets):
    from concourse.bass_utils import run_bass_kernel_spmd

    nc = _get_nc()
    in_maps = _prep_inputs(x, shapelets)
    res = run_bass_kernel_spmd(nc, in_maps, core_ids=list(range(NCORES)))
    return _gather(res.results)


def kernel_traced(x, shapelets):
    """Like kernel() but requests an NTFF trace; returns (out, BassKernelResults)."""
    from concourse.bass_utils import run_bass_kernel_spmd

    nc = _get_nc()
    in_maps = _prep_inputs(x, shapelets)
    res = run_bass_kernel_spmd(nc, in_maps, core_ids=list(range(NCORES)), trace=True)
    return _gather(res.results), res


# revision 15
# speedup vs baseline: 1.4711x; 1.4605x over previous
"""Trainium2 Bass kernel for nn_MinEuclideanDistBlock.

Math (reference):
  x: (B=64, C=3, L=2048), shapelets: (C=3, N=256, S=64)
  W = L - S + 1 = 1985 sliding windows
  d2[b,c,w,n] = |win|^2 + |shp|^2 - 2 win.shp    (win = x[b,c,w:w+S])
  d = sqrt(max(d2, 0));  out[b,0,n] = min_w sum_c d[b,c,w,n]

Device strategy (per core, batch-sharded B/8 = 8 batches per core):
  - For each (b, c): build a "shifted" SBUF tile T[65, 2048]:
      T[s, w] = x[b, c, w+s] for s in [0,64)   (one DMA, overlapping AP)
      T[64, w] = win_sq[w] = sum_s x[b,c,w+s]^2 (computed on-chip via
      log2 shifted adds on DVE, row-copied via DMA)
  - lhsT[k, n] (host precomputed): rows 0..63 = -2*shapelets^T, row 64 = 1.
    One float32r matmul per 512-wide w chunk gives
      psum[n, w] = -2*cross + win_sq[w]
  - ACT: d = sqrt(psum + bias) with per-partition bias = shp_sq[n].
  - DVE: accumulate channels, then tensor_tensor_reduce(min over w).
"""

import numpy as np

S = 64
NSH = 256
C = 3
B = 64
L = 2048
W = L - S + 1  # 1985
NCORES = 8
BPC = B // NCORES  # 8
NT = 2  # shapelet tiles of 128
WCHUNKS = [(0, 512), (512, 512), (1024, 512), (1536, W - 1536)]

_cache = {}


def _build_nc(reps=1, ablate=()):
    from contextlib import ExitStack  # noqa: F401

    import concourse.bass as bass
    import concourse.bacc as bacc
    import concourse.mybir as mybir
    import concourse.tile as tile

    f32 = mybir.dt.float32
    bf16 = mybir.dt.bfloat16
    AF = mybir.ActivationFunctionType
    ALU = mybir.AluOpType

    nc = bacc.Bacc()
    xs = nc.dram_tensor("xs", [BPC, C, L], bf16, kind="ExternalInput")
    wts = nc.dram_tensor("wts", [C, NT, S + 1, 128], bf16, kind="ExternalInput")
    ssq = nc.dram_tensor("ssq", [C, NT, 128], f32, kind="ExternalInput")
    out = nc.dram_tensor("out", [BPC, NT, 128], f32, kind="ExternalOutput")

    with tile.TileContext(nc) as tc:
        with (
            tc.tile_pool(name="consts", bufs=1) as consts,
            tc.tile_pool(name="prep", bufs=1) as prep,
            tc.tile_pool(name="tpool", bufs=6) as tpool,
            tc.tile_pool(name="psump", bufs=2, space="PSUM") as psump,
            tc.tile_pool(name="accp", bufs=2) as accp,
            tc.tile_pool(name="tmpp", bufs=3) as tmpp,
            tc.tile_pool(name="minvp", bufs=8) as minvp,
        ):
            # ---- constants ----
            w_all = consts.tile([S + 1, C * NT * 128], bf16)
            biases = {}
            for c in range(C):
                for nt in range(NT):
                    idx = c * NT + nt
                    nc.sync.dma_start(
                        out=w_all[:, idx * 128 : (idx + 1) * 128],
                        in_=wts[c, nt, :, :],
                    )
                    bt = consts.tile([128, 1], f32, name=f"bias_{c}_{nt}")
                    nc.sync.dma_start(out=bt, in_=ssq[c, nt, :])
                    biases[(c, nt)] = bt

            # ---- prep: win_sq for all 24 (b, c) rows ----
            for _rep in range(reps):
                _body(nc, tc, bass, mybir, prep, tpool, psump, accp, tmpp, minvp,
                      xs, out, w_all, biases, ablate)
    nc.compile()
    return nc


def _body(nc, tc, bass, mybir, prep, tpool, psump, accp, tmpp, minvp,
          xs, out, w_all, biases, ablate=()):
    f32 = mybir.dt.float32
    bf16 = mybir.dt.bfloat16
    AF = mybir.ActivationFunctionType
    ALU = mybir.AluOpType
    if True:
        if True:
            xr = prep.tile([BPC * C, L], bf16)
            nc.sync.dma_start(out=xr, in_=xs[:, :, :])
            xsq = prep.tile([BPC * C, L], f32)
            nc.vector.tensor_mul(xsq, xr, xr)
            msA = prep.tile([BPC * C, L], f32)
            msB = prep.tile([BPC * C, L], f32)
            winsq = prep.tile([BPC * C, L], bf16)
            bufs = [xsq, msA, msB]
            srci = 0
            cnt = L
            for step_i, k in enumerate([1, 2, 4, 8, 16, 32]):
                src = bufs[srci]
                cnt = cnt - k
                if k == 32:  # last step writes the bf16 win_sq row directly
                    dst = winsq
                else:
                    dst = bufs[1 + (step_i % 2)]
                nc.vector.tensor_add(
                    dst[:, 0:cnt], src[:, 0:cnt], src[:, k : k + cnt]
                )
                srci = 1 + (step_i % 2)
            assert cnt == W

            # ---- main loop ----
            accs = {}
            T_shared = None
            for b in range(BPC):
                for c in range(C):
                    bc = b * C + c
                    if "t_reuse" in ablate:
                        # ablation: one T per batch (wrong data for c>0)
                        build_T = c == 0
                        if build_T:
                            T_shared = tpool.tile([S + 1, L], bf16, name="T")
                        T = T_shared
                    else:
                        build_T = True
                        T = tpool.tile([S + 1, L], bf16, name="T")
                    if build_T:
                        base = xs[b, c, :]
                        apov = bass.AP(
                            tensor=base.tensor,
                            offset=base.offset,
                            ap=[[1, S], [1, W]],
                        )
                        nc.sync.dma_start(out=T[0:S, 0:W], in_=apov)
                        nc.sync.dma_start(
                            out=T[S : S + 1, 0:W], in_=winsq[bc : bc + 1, 0:W]
                        )
                    AW = 992 if "halfw" in ablate else W
                    for nt in range(NT):
                        idx = c * NT + nt
                        lhsT = w_all[:, idx * 128 : (idx + 1) * 128]
                        ps = psump.tile([128, 2048], f32, name="ps")
                        if "nomm" not in ablate:
                            for w0, wl in WCHUNKS:
                                nc.tensor.matmul(
                                    ps[:, w0 : w0 + wl],
                                    lhsT=lhsT,
                                    rhs=T[:, w0 : w0 + wl],
                                    start=True,
                                    stop=True,
                                )
                        if c == 0:
                            acc = accp.tile([128, W], f32, name=f"acc{nt}")
                            accs[nt] = acc
                            nc.scalar.activation(
                                acc[:, 0:AW], ps[:, 0:AW], AF.Sqrt,
                                bias=biases[(c, nt)]
                            )
                        elif c == 1:
                            tmp = tmpp.tile([128, W], f32, name="tmp")
                            nc.scalar.activation(
                                tmp[:, 0:AW], ps[:, 0:AW], AF.Sqrt,
                                bias=biases[(c, nt)]
                            )
                            # run this add on the otherwise-idle GPSIMD so the
                            # DVE only carries one add + the min-reduce
                            nc.gpsimd.tensor_add(
                                accs[nt][:, 0:AW], accs[nt][:, 0:AW], tmp[:, 0:AW]
                            )
                        else:
                            tmp = tmpp.tile([128, W], f32, name="tmp")
                            nc.scalar.activation(
                                tmp[:, 0:AW], ps[:, 0:AW], AF.Sqrt,
                                bias=biases[(c, nt)]
                            )
                            scratch = tmpp.tile([128, W], f32, name="scratch")
                            minv = minvp.tile([128, 1], f32, name="minv")
                            nc.vector.tensor_add(
                                scratch[:, 0:AW], accs[nt][:, 0:AW], tmp[:, 0:AW]
                            )
                            nc.vector.tensor_reduce(
                                minv,
                                scratch[:, 0:AW],
                                mybir.AxisListType.X,
                                ALU.min,
                            )
                            nc.sync.dma_start(out=out[b, nt, :], in_=minv)


def _get_nc():
    if "nc" not in _cache:
        _cache["nc"] = _build_nc()
    return _cache["nc"]


def _prep_inputs(x, shapelets):
    import ml_dtypes

    bf16 = ml_dtypes.bfloat16
    x = np.ascontiguousarray(np.asarray(x), dtype=np.float32)
    sh = np.asarray(shapelets, dtype=np.float32)
    # round shapelets to bf16 once; all derived quantities use the rounded
    # values so d2 stays an exact squared distance of the rounded vectors
    shb = sh.astype(bf16).astype(np.float32)
    shT = np.transpose(shb, (0, 2, 1))  # (C, S, N)
    wts = np.empty((C, NT, S + 1, 128), np.float32)
    for nt in range(NT):
        wts[:, nt, :S, :] = -2.0 * shT[:, :, nt * 128 : (nt + 1) * 128]
    wts[:, :, S, :] = 1.0
    ssq = np.sum(shb * shb, axis=2).reshape(C, NT, 128).astype(np.float32)
    wts_b = np.ascontiguousarray(wts.astype(bf16))
    ssq = np.ascontiguousarray(ssq)
    xb = x.astype(bf16)
    in_maps = [
        {
            "xs": np.ascontiguousarray(xb[k * BPC : (k + 1) * BPC]),
            "wts": wts_b,
            "ssq": ssq,
        }
        for k in range(NCORES)
    ]
    return in_maps


def _gather(results):
    outs = [np.asarray(r["out"]).reshape(BPC, NSH) for r in results]
    full = np.concatenate(outs, axis=0)  # (64, 256)
    return np.ascontiguousarray(full[:, None, :]).astype(np.float32)  # (64, 1, 256)


def kernel(x, shapelets):
    from concourse.bass_utils import run_bass_kernel_spmd

    nc = _get_nc()
    in_maps = _prep_inputs(x, shapelets)
    res = run_bass_kernel_spmd(nc, in_maps, core_ids=list(range(NCORES)))
    return _gather(res.results)


def kernel_traced(x, shapelets):
    """Like kernel() but requests an NTFF trace; returns (out, BassKernelResults)."""
    from concourse.bass_utils import run_bass_kernel_spmd

    nc = _get_nc()
    in_maps = _prep_inputs(x, shapelets)
    res = run_bass_kernel_spmd(nc, in_maps, core_ids=list(range(NCORES)), trace=True)
    return _gather(res.results), res
